# revision 1
# baseline (speedup 1.0000x reference)
"""MultiHeadAttention Trainium2 kernel (8 NeuronCores).

Sharding: core c handles batch b = c // 2 and head-group hg = c % 2
(8 of 16 heads, 512 of 1024 model dims). Attention is embarrassingly
parallel over (b, hg); the output projection is computed per head-group
against the matching W_o columns, yielding partial outputs that the host
sums (plus b_o).

Device dataflow (per core), all in "transposed" layouts so no on-device
transposes are ever needed:
  qT = Wq_hg @ Xq^T      [dh=512, S]   (lhsT = Wq_hg^T, rhs = Xq^T)
  kT = Wk_hg @ Xk^T      [dh=512, S]
  v  = Xv @ Wv_hg^T      [S, dh=512]   (+ ones column per head for sums)
  scores_T[k, q]: per head-pair (m) and half (hl), keys on partitions;
    both hl matmuls write one 2-bank PSUM tile [128, 2, QB] so a single
    fused ACT exp covers the pair. Diagonal chunks restrict columns to
    the causally-reachable range; the 128-col triangular block is zeroed
    after exp with one small Pool-engine multiply.
  probs -> PV: attn_T[d, q] + sums row accumulated in PSUM [65, QB],
    rhs column-restricted per chunk (no memsets needed).
  normalize: reciprocal of the sums row in-place at partition 64 (DVE),
    ones-matmul broadcast to [64, QB], multiply (DVE). No DMA round-trips.
  out_partial = attn^T-matmul with Wo columns
"""

import os

import numpy as np

B, S_FULL, D = 4, 2048, 1024
H, DK = 16, 64
NH_G = 8          # heads per core
DH = NH_G * DK    # 512 dims per core
P = 128
KC = 128          # key chunk (PE contraction)
SCALE = 1.0 / np.sqrt(np.float32(DK))

_PROG_CACHE = {}


def _dims(S):
    QB = min(512, S)
    return {
        "S": S, "QB": QB, "N_QB": S // QB, "N_KC": S // KC,
        "R": QB // KC, "E_CH": D // P, "M_CH": DH // P, "O_N": D // 512,
    }


def _np_dt(use_bf16):
    if use_bf16:
        import ml_dtypes
        return ml_dtypes.bfloat16
    return np.float32


def build_program(causal, S, use_bf16=True):
    """Build the single-core Bass/Tile program (same program on all 8 cores)."""
    from contextlib import ExitStack

    import concourse.bass as bass
    import concourse.tile as tile
    from concourse import bacc, mybir

    d = _dims(S)
    QB, N_QB, N_KC, R, E_CH, M_CH, O_N = (
        d["QB"], d["N_QB"], d["N_KC"], d["R"], d["E_CH"], d["M_CH"], d["O_N"])

    DT = mybir.dt.bfloat16 if use_bf16 else mybir.dt.float32r
    F32 = mybir.dt.float32
    F32R = mybir.dt.float32r
    AF = mybir.ActivationFunctionType
    ALU = mybir.AluOpType

    WB = QB
    NW = S // WB

    nc = bacc.Bacc("TRN2", target_bir_lowering=False, debug=False)

    NB = S // QB
    xq_t = nc.dram_tensor("xq_t", [NB, P, E_CH, QB], DT,
                          kind="ExternalInput").ap()
    xk_t = nc.dram_tensor("xk_t", [NB, P, E_CH, QB], DT,
                          kind="ExternalInput").ap()
    xv_t = nc.dram_tensor("xv_t", [NB, P, E_CH, QB], DT,
                          kind="ExternalInput").ap()
    wq_t = nc.dram_tensor("wq_t", [P, E_CH, DH], DT,
                          kind="ExternalInput").ap()
    wk_t = nc.dram_tensor("wk_t", [P, E_CH, DH], DT,
                          kind="ExternalInput").ap()
    wv_t = nc.dram_tensor("wv_t", [P, E_CH, DH], DT,
                          kind="ExternalInput").ap()
    wo_t = nc.dram_tensor("wo_t", [P, M_CH, D], DT,
                          kind="ExternalInput").ap()
    bq_in = nc.dram_tensor("bq_p", [P, M_CH], F32, kind="ExternalInput").ap()
    bk_in = nc.dram_tensor("bk_p", [P, M_CH], F32, kind="ExternalInput").ap()
    bv_in = nc.dram_tensor("bv_r", [P, DH], F32, kind="ExternalInput").ap()
    dmask_in = nc.dram_tensor("dmask", [P, KC], DT,
                              kind="ExternalInput").ap()
    ones_c_in = nc.dram_tensor("ones_c", [65, 64], DT,
                               kind="ExternalInput").ap()
    ones_v_in = nc.dram_tensor("ones_v", [P, N_KC, NH_G, 1], DT,
                               kind="ExternalInput").ap()
    out_p = nc.dram_tensor("out_p", [S, D], F32, kind="ExternalOutput").ap()

    with tile.TileContext(nc) as tc, ExitStack() as ctx:
        consts = ctx.enter_context(tc.tile_pool(name="consts", bufs=1))
        wpool = ctx.enter_context(tc.tile_pool(name="w", bufs=2))
        qkv = ctx.enter_context(tc.tile_pool(name="qkv", bufs=1))

        bq_sb = consts.tile([P, M_CH], F32)
        nc.sync.dma_start(bq_sb, bq_in)
        bk_sb = consts.tile([P, M_CH], F32)
        nc.sync.dma_start(bk_sb, bk_in)
        bv_sb = consts.tile([P, DH], F32)
        nc.sync.dma_start(bv_sb, bv_in)

        qT = qkv.tile([P, M_CH, S], DT, tag="qT")
        kT = qkv.tile([P, M_CH, S], DT, tag="kT")
        v_aug = qkv.tile([P, N_KC, NH_G, 65], DT, tag="v_aug")
        if use_bf16:
            nc.gpsimd.memset(v_aug[:, :, :, 64:65], 1.0)
        else:
            nc.gpsimd.dma_start(v_aug[:, :, :, 64:65], ones_v_in)
        w_tiles = {}
        for name in ("wq", "wk", "wv"):
            w_tiles[name] = wpool.tile([P, E_CH, DH], DT, tag="w",
                                       name=name)
        wo_sb = wpool.tile([P, M_CH, D], DT, tag="w")

        # attention-phase SBUF pools + constants (needed by the qb=0
        # prologue that runs interleaved with the projections)
        probs_pool = ctx.enter_context(tc.tile_pool(name="probs", bufs=4))
        attn_pool = ctx.enter_context(
            tc.tile_pool(name="attn", bufs=M_CH + 1))
        misc = ctx.enter_context(tc.tile_pool(name="misc", bufs=4))
        aupool = ctx.enter_context(
            tc.tile_pool(name="aupool", bufs=2 * M_CH + 3))
        outst = ctx.enter_context(tc.tile_pool(name="outst", bufs=3))
        tri_sb = consts.tile([P, KC], DT)
        nc.sync.dma_start(tri_sb, dmask_in)
        ones65 = consts.tile([65, 64], DT)
        nc.sync.dma_start(ones65, ones_c_in)

        def attn_m_group(qb, m, sc_pool, pvp, sums_g, mq_work):
            """scores+exp+PV+drain for one (qb, m); yields once per chunk
            so the caller can interleave other work into the PE queue."""
            n_kc = (qb + 1) * R if causal else N_KC
            pv_t = [pvp.tile([65, 512], F32, tag="pv", name=f"pv{hl}")
                    for hl in (0, 1)]

            def emit_pv(kc, pt, c0):
                for hl in (0, 1):
                    nc.tensor.matmul(
                        pv_t[hl][:, c0:QB],
                        lhsT=v_aug[:, kc, 2 * m + hl, :],
                        rhs=pt[:, hl, c0:],
                        start=(kc == 0), stop=(kc == n_kc - 1),
                    )

            prev = None
            for kc in range(n_kc):
                r = kc - (n_kc - R) if causal else -1
                c0 = KC * r if r > 0 else 0
                sc2 = sc_pool.tile([P, 2, 512], F32, tag="sc", name="sc2")
                for hl in (0, 1):
                    rows = slice(64 * hl, 64 * hl + 64)
                    nc.tensor.matmul(
                        sc2[:, hl, c0:QB],
                        lhsT=kT[rows, m, kc * KC:(kc + 1) * KC],
                        rhs=qT[rows, m, qb * QB + c0:(qb + 1) * QB],
                        start=True, stop=True,
                    )
                pt = probs_pool.tile([P, 2, QB], DT, tag="pt")
                nc.scalar.activation(pt[:, :, c0:], sc2[:, :, c0:QB],
                                     AF.Exp, scale=float(SCALE))
                if r >= 0:
                    for hl in (0, 1):
                        nc.gpsimd.tensor_tensor(
                            pt[:, hl, c0:c0 + KC], pt[:, hl, c0:c0 + KC],
                            tri_sb, ALU.mult)
                if prev is not None:
                    emit_pv(*prev)
                prev = (kc, pt, c0)
                yield
            emit_pv(*prev)
            attn_us = []
            for hl in (0, 1):
                sums_sb = misc.tile([65, QB], F32, tag="sums_sb",
                                    name=f"sums{hl}")
                nc.any.tensor_copy(sums_sb[64:65, :], pv_t[hl][64:65, 0:QB])
                nc.sync.dma_start(sums_g[2 * m + hl: 2 * m + hl + 1],
                                  sums_sb[64:65, :])
                attn_u = aupool.tile([64, QB], DT, tag="attn_u",
                                     name=f"attn_u{hl}")
                nc.any.tensor_copy(attn_u, pv_t[hl][0:64, 0:QB])
                attn_us.append(attn_u)
            mq_work.append((m, attn_us))
            yield

        def emit_recip(nsums_g, nrecips_f, nrecips_g):
            # one 8-lane reciprocal for all (m, hl) of a qb, then a
            # downcast so the broadcast matmul runs in bf16
            with nc.allow_low_precision(
                    reason="softmax denom recip, f32r rounding"):
                nc.vector.reciprocal(nrecips_f, nsums_g)
            nc.vector.tensor_copy(nrecips_g, nrecips_f)

        def norm_outproj(nqb, nmq, nsums_g, nrecips_f, nrecips_g, rbop,
                         recip_done=False):
            attn_tiles = []
            for m, attn_us in nmq:
                if m == 0 and not recip_done:
                    emit_recip(nsums_g, nrecips_f, nrecips_g)
                attn_m = attn_pool.tile([P, QB], DT, tag="attn",
                                        name="attn_m")
                for hl in (0, 1):
                    recip65 = misc.tile([65, QB], DT, tag="recip",
                                        name="recip65")
                    nc.gpsimd.dma_start(
                        recip65[64:65, :],
                        nrecips_g[2 * m + hl: 2 * m + hl + 1])
                    rb = rbop.tile([P, 512], F32, tag="rbop", name="rb")
                    nc.tensor.matmul(rb[0:64, 0:QB],
                                     lhsT=ones65[64:65, :],
                                     rhs=recip65[64:65, :],
                                     start=True, stop=True)
                    nc.vector.tensor_tensor(
                        attn_m[64 * hl:64 * hl + 64, :], attn_us[hl],
                        rb[0:64, 0:QB], ALU.mult)
                attn_tiles.append(attn_m)
            for ssub in range(QB // P):
                for nout in range(O_N):
                    pso = rbop.tile([P, 512], F32, tag="rbop", name="pso")
                    for m in range(M_CH):
                        nc.tensor.matmul(
                            pso,
                            lhsT=attn_tiles[m][:, ssub * P:(ssub + 1) * P],
                            rhs=wo_sb[:, m, nout * 512:(nout + 1) * 512],
                            start=(m == 0), stop=(m == M_CH - 1),
                        )
                    st = outst.tile([P, 512], F32, tag="st", name="st")
                    nc.any.tensor_copy(st, pso)
                    nc.gpsimd.dma_start(
                        out_p[nqb * QB + ssub * P: nqb * QB + (ssub + 1) * P,
                              nout * 512:(nout + 1) * 512],
                        st)

        # ---- projections + qb=0 attention prologue ----
        # PSUM: pj 2 + pjv 2 + scp 2 + pvp 2 = 8 banks. The prologue's
        # attention uses scp/pvp so it can be EMITTED interleaved with the
        # k/q projections of blocks 1..NW-1 (in-order engine queues make
        # emission order the real schedule).
        mq0 = []
        sums_g0 = recips_f0 = recips_g0 = None
        with tc.tile_pool(name="xp", bufs=3) as xpool, \
             tc.tile_pool(name="pj", bufs=2, space="PSUM") as pj_ps, \
             tc.tile_pool(name="pjv", bufs=2, space="PSUM") as pjv_ps, \
             tc.tile_pool(name="scp", bufs=1, space="PSUM") as scp_ps, \
             tc.tile_pool(name="pvp", bufs=2, space="PSUM") as pvp_ps:
            # v projection first (v_aug ready before any PV)
            for e in range(E_CH):
                nc.sync.dma_start(w_tiles["wv"][:, e], wv_t[:, e])
            for n in range(NB):
                xblk = xpool.tile([P, E_CH, QB], DT, tag="x")
                for e in range(E_CH):
                    nc.sync.dma_start(xblk[:, e], xv_t[n, :, e])
                for sc in range(QB // P):
                    ps = pjv_ps.tile([P, DH], F32, tag="pjv")
                    for e in range(E_CH):
                        nc.tensor.matmul(
                            ps,
                            lhsT=xblk[:, e, sc * P:(sc + 1) * P],
                            rhs=w_tiles["wv"][:, e, :],
                            start=(e == 0), stop=(e == E_CH - 1),
                        )
                    kc = n * (QB // P) + sc
                    nc.vector.tensor_tensor(
                        v_aug[:, kc, :, 0:64],
                        ps.rearrange("p (h e) -> p h e", h=NH_G),
                        bv_sb.rearrange("p (h e) -> p h e", h=NH_G),
                        ALU.add,
                    )

            for name2, srcw in (("wk", wk_t), ("wq", wq_t)):
                for e in range(E_CH):
                    nc.sync.dma_start(w_tiles[name2][:, e], srcw[:, e])

            def kq_load(phase, n2):
                x_in = xk_t if phase == "k" else xq_t
                xblk = xpool.tile([P, E_CH, WB], DT, tag="x2", name="xblk")
                for e in range(E_CH):
                    nc.sync.dma_start(xblk[:, e], x_in[n2, :, e])
                return xblk

            def kq_group(phase, n2, m, xblk):
                w_sb = w_tiles["wk" if phase == "k" else "wq"]
                b_sb = bk_sb if phase == "k" else bq_sb
                ps = pj_ps.tile([P, WB], F32, tag="pj", name="ps")
                for e in range(E_CH):
                    nc.tensor.matmul(
                        ps,
                        lhsT=w_sb[:, e, m * P:(m + 1) * P],
                        rhs=xblk[:, e, :],
                        start=(e == 0), stop=(e == E_CH - 1),
                    )
                dstp = kT if phase == "k" else qT
                nc.vector.tensor_scalar_add(
                    dstp[:, m, n2 * WB:(n2 + 1) * WB], ps, b_sb[:, m:m + 1])

            # k and q block 0 straight through; qb=0 needs only these
            for phase in ("k", "q"):
                xb = kq_load(phase, 0)
                for m in range(M_CH):
                    kq_group(phase, 0, m, xb)
            nc.sync.dma_start(wo_sb, wo_t)

            if causal:
                sums_g0 = misc.tile([2 * M_CH, QB], F32, tag="sums_g",
                                    name="sums_g0")
                recips_f0 = misc.tile([2 * M_CH, QB], F32R, tag="recips_f",
                                      name="rf0")
                recips_g0 = misc.tile([2 * M_CH, QB], DT, tag="recips_g",
                                      name="rg0")

                mq1 = []
                sums_g1 = misc.tile([2 * M_CH, QB], F32, tag="sums_g",
                                    name="sums_g1")
                recips_f1 = misc.tile([2 * M_CH, QB], F32R, tag="recips_f",
                                      name="rf1")
                recips_g1 = misc.tile([2 * M_CH, QB], DT, tag="recips_g",
                                      name="rg1")

                def prologue():
                    for m in range(M_CH):
                        yield from attn_m_group(0, m, scp_ps, pvp_ps,
                                                sums_g0, mq0)
                    if N_QB > 1:
                        # spare fill capacity: qb=1's first m-group too
                        yield from attn_m_group(1, 0, scp_ps, pvp_ps,
                                                sums_g1, mq1)
                gen = prologue()
            else:
                gen = iter(())

            # remaining k/q blocks, one prologue chunk per psum group so
            # the PE fills the prologue's exp-wait gaps with projection work
            for n2 in range(1, NW):
                for phase in ("k", "q"):
                    xb = kq_load(phase, n2)
                    for m in range(M_CH):
                        kq_group(phase, n2, m, xb)
                        next(gen, None)
            for _ in gen:
                pass
            if causal:
                # qb=0's recip computed inside the projection window so the
                # main section's first rb matmuls never wait on it
                emit_recip(sums_g0, recips_f0, recips_g0)

        # ---- main attention + output projection ----
        # PSUM: sc2 2 bufs x 2 banks + pv 2 x 1 bank + rbop 2 x 1 bank = 8
        with tc.tile_pool(name="sc_ps", bufs=2, space="PSUM") as sc_ps, \
             tc.tile_pool(name="pv_ps", bufs=2, space="PSUM") as pv_pool, \
             tc.tile_pool(name="rbop", bufs=2, space="PSUM") as rbop:
            if causal:
                norm_outproj(0, mq0, sums_g0, recips_f0, recips_g0, rbop,
                             recip_done=True)
            for qb in (range(1, N_QB) if causal else range(N_QB)):
                if causal and qb == 1:
                    mq_work = mq1
                    sums_g, recips_f, recips_g = sums_g1, recips_f1, recips_g1
                    m_start = 1
                else:
                    mq_work = []
                    sums_g = misc.tile([2 * M_CH, QB], F32, tag="sums_g")
                    recips_f = misc.tile([2 * M_CH, QB], F32R,
                                         tag="recips_f")
                    recips_g = misc.tile([2 * M_CH, QB], DT, tag="recips_g")
                    m_start = 0
                for m in range(m_start, M_CH):
                    for _ in attn_m_group(qb, m, sc_ps, pv_pool, sums_g,
                                          mq_work):
                        pass
                norm_outproj(qb, mq_work, sums_g, recips_f, recips_g, rbop)
    nc.compile()
    return nc


def make_consts(S, use_bf16):
    """Host-built 0/1 upper-triangular (key<=query) mask for diag chunks."""
    npdt = _np_dt(use_bf16)
    i = np.arange(P)[:, None]
    j = np.arange(KC)[None, :]
    return (i <= j).astype(npdt)


def core_inputs(Q, K, V, W_q, b_q, W_k, b_k, W_v, b_v, W_o, b, hg, S, use_bf16):
    """Build the per-core input map (host-side slicing/transposition/casts)."""
    npdt = _np_dt(use_bf16)
    d = _dims(S)
    M_CH = d["M_CH"]
    rows = slice(hg * DH, (hg + 1) * DH)

    QB = d["QB"]
    E_CH = D // P

    def xt(x):
        # [S, D] -> [N_QB, P, E_CH, QB]: per-core x, transposed and tiled so
        # each device block load is a contiguous DMA.
        a = np.asarray(x, np.float32).T.astype(npdt)      # [D, S]
        a = a.reshape(E_CH, P, S // QB, QB).transpose(2, 1, 0, 3)
        return np.ascontiguousarray(a)

    def wt(w):
        # [DH, D] slice -> W^T tiled [P, E_CH, DH]
        a = np.asarray(w, np.float32).T.astype(npdt)      # [D, DH]
        return np.ascontiguousarray(
            a.reshape(E_CH, P, DH).transpose(1, 0, 2))

    a_wo = np.asarray(W_o[:, rows], np.float32).T.astype(npdt)  # [DH, D]
    wo_prep = np.ascontiguousarray(
        a_wo.reshape(M_CH, P, D).transpose(1, 0, 2))

    dmask = make_consts(S, use_bf16)
    return {
        "xq_t": xt(Q[b]), "xk_t": xt(K[b]), "xv_t": xt(V[b]),
        "wq_t": wt(W_q[rows]), "wk_t": wt(W_k[rows]), "wv_t": wt(W_v[rows]),
        "wo_t": wo_prep,
        "bq_p": np.ascontiguousarray(
            np.asarray(b_q[rows], np.float32).reshape(M_CH, P).T),
        "bk_p": np.ascontiguousarray(
            np.asarray(b_k[rows], np.float32).reshape(M_CH, P).T),
        "bv_r": np.broadcast_to(
            np.asarray(b_v[rows], np.float32), (P, DH)).copy(),
        "dmask": dmask,
        "ones_c": np.ones((65, 64), npdt),
        "ones_v": np.ones((P, d["N_KC"], NH_G, 1), npdt),
    }


def _np_reference(Q, K, V, mask, W_q, b_q, W_k, b_k, W_v, b_v, W_o, b_o):
    """Exact numpy fallback for arbitrary masks."""
    q = (Q @ W_q.T + b_q).reshape(B, S_FULL, H, DK).transpose(0, 2, 1, 3)
    k = (K @ W_k.T + b_k).reshape(B, S_FULL, H, DK).transpose(0, 2, 1, 3)
    v = (V @ W_v.T + b_v).reshape(B, S_FULL, H, DK).transpose(0, 2, 1, 3)
    scores = np.einsum("bhqd,bhkd->bhqk", q, k) / np.sqrt(np.float32(DK))
    scores = np.where(mask == 0, np.finfo(np.float32).min, scores)
    scores -= scores.max(-1, keepdims=True)
    probs = np.exp(scores)
    probs /= probs.sum(-1, keepdims=True)
    out = np.einsum("bhqk,bhkd->bhqd", probs, v)
    out = out.transpose(0, 2, 1, 3).reshape(B, S_FULL, D)
    return (out @ W_o.T + b_o).astype(np.float32)


def kernel(Q, K, V, mask, W_q, b_q, W_k, b_k, W_v, b_v, W_o, b_o):
    Q = np.asarray(Q, np.float32)
    K = np.asarray(K, np.float32)
    V = np.asarray(V, np.float32)
    mask = np.asarray(mask)

    m2 = mask.reshape(mask.shape[-2], mask.shape[-1])
    if np.array_equal(m2 != 0, np.tril(np.ones(m2.shape, bool))):
        causal = True
    elif (m2 != 0).all():
        causal = False
    else:
        return _np_reference(Q, K, V, mask, W_q, b_q, W_k, b_k, W_v, b_v,
                             W_o, b_o)

    use_bf16 = os.environ.get("MHA_KERNEL_DTYPE", "bf16") == "bf16"
    from concourse.bass_utils import run_bass_kernel_spmd

    key = (causal, S_FULL, use_bf16)
    if key not in _PROG_CACHE:
        _PROG_CACHE[key] = build_program(causal, S_FULL, use_bf16)
    nc = _PROG_CACHE[key]

    in_maps = []
    for c in range(8):
        b, hg = divmod(c, 2)
        in_maps.append(core_inputs(Q, K, V, W_q, b_q, W_k, b_k, W_v, b_v,
                                   W_o, b, hg, S_FULL, use_bf16))

    trace = os.environ.get("MHA_KERNEL_TRACE", "0") == "1"
    kw = {}
    if trace:
        kw = {"trace": True,
              "trace_cores": [int(x) for x in os.environ.get(
                  "MHA_TRACE_CORES", "0").split(",")]}
    n_cores = int(os.environ.get("MHA_CORES", "8"))
    res = run_bass_kernel_spmd(nc, in_maps[:n_cores],
                               core_ids=list(range(n_cores)), **kw)
    kernel.last_results = res

    b_o32 = np.asarray(b_o, np.float32)
    out = np.zeros((B, S_FULL, D), np.float32)
    for b in range(B):
        if 2 * b + 1 < n_cores:
            out[b] = (res.results[2 * b]["out_p"]
                      + res.results[2 * b + 1]["out_p"] + b_o32[None, :])
    return out


kernel.last_results = None



# revision 12
# speedup vs baseline: 1.1487x; 1.1487x over previous
"""MultiHeadAttention Trainium2 kernel (8 NeuronCores).

Sharding: core c handles batch b = c // 2 and head-group hg = c % 2
(8 of 16 heads, 512 of 1024 model dims). Attention is embarrassingly
parallel over (b, hg); the output projection is computed per head-group
against the matching W_o columns, yielding partial outputs that the host
sums (plus b_o).

Device dataflow (per core), all in "transposed" layouts so no on-device
transposes are ever needed:
  qT = Wq_hg @ Xq^T      [dh=512, S]   (lhsT = Wq_hg^T, rhs = Xq^T)
  kT = Wk_hg @ Xk^T      [dh=512, S]
  v  = Xv @ Wv_hg^T      [S, dh=512]   (+ ones column per head for sums)
  scores_T[k, q]: per head-pair (m) and half (hl), keys on partitions;
    the two hl matmuls run CONCURRENTLY as 64-row PE tiles (row_grp h0 /
    h64, auto-derived from lhsT base partitions) and write one 2-bank
    PSUM tile [128, 2, QB] so a single fused ACT exp covers the pair.
    Diagonal chunks restrict columns to the causally-reachable range;
    the 128-col triangular block is zeroed after exp with one small
    Pool-engine multiply.
  probs -> PV: attn_T[d, q] + sums row accumulated in PSUM [65, QB],
    rhs column-restricted per chunk.
  normalize: one reciprocal_approx_fast over the gathered sums rows,
    then a GPSIMD partition_broadcast of each recip row to 64 partitions
    and a DVE multiply (no PE broadcast matmuls, no DMA round-trips).
  out_partial = attn^T-matmul with Wo columns.

Schedule (emission order == engine-queue order):
  V proj | K0 Q0 | K1..K3 Q1..Q3 interleaved with qb0+qb1 attention
  (generators yield per key-chunk so exp/PV ride the projection window)
  | qb0 norm+outproj interleaved with the qb1 attention drain
  | for qb in 2..: attn(qb) m-groups interleaved with norm+outproj(qb-1)
  | norm+outproj(last).
"""

import os

import numpy as np

B, S_FULL, D = 4, 2048, 1024
H, DK = 16, 64
NH_G = 8          # heads per core
DH = NH_G * DK    # 512 dims per core
P = 128
KC = 128          # key chunk (PE contraction)
SCALE = 1.0 / np.sqrt(np.float32(DK))

_PROG_CACHE = {}


def _dims(S):
    QB = min(512, S)
    return {
        "S": S, "QB": QB, "N_QB": S // QB, "N_KC": S // KC,
        "R": QB // KC, "E_CH": D // P, "M_CH": DH // P, "O_N": D // 512,
    }


def _np_dt(use_bf16):
    if use_bf16:
        import ml_dtypes
        return ml_dtypes.bfloat16
    return np.float32


def build_program(causal, S, use_bf16=True):
    """Build the single-core Bass/Tile program (same program on all 8 cores)."""
    from contextlib import ExitStack

    import concourse.bass as bass
    import concourse.tile as tile
    from concourse import bacc, mybir

    d = _dims(S)
    QB, N_QB, N_KC, R, E_CH, M_CH, O_N = (
        d["QB"], d["N_QB"], d["N_KC"], d["R"], d["E_CH"], d["M_CH"], d["O_N"])

    DT = mybir.dt.bfloat16 if use_bf16 else mybir.dt.float32r
    F32 = mybir.dt.float32
    AF = mybir.ActivationFunctionType
    ALU = mybir.AluOpType

    WB = QB
    NW = S // WB

    nc = bacc.Bacc("TRN2", target_bir_lowering=False, debug=False)

    NB = S // QB
    xq_t = nc.dram_tensor("xq_t", [NB, P, E_CH, QB], DT,
                          kind="ExternalInput").ap()
    xk_t = nc.dram_tensor("xk_t", [NB, P, E_CH, QB], DT,
                          kind="ExternalInput").ap()
    xv_t = nc.dram_tensor("xv_t", [NB, P, E_CH, QB], DT,
                          kind="ExternalInput").ap()
    wq_t = nc.dram_tensor("wq_t", [P, E_CH, DH], DT,
                          kind="ExternalInput").ap()
    wk_t = nc.dram_tensor("wk_t", [P, E_CH, DH], DT,
                          kind="ExternalInput").ap()
    wv_t = nc.dram_tensor("wv_t", [P, E_CH, DH], DT,
                          kind="ExternalInput").ap()
    wo_t = nc.dram_tensor("wo_t", [P, M_CH, D], DT,
                          kind="ExternalInput").ap()
    bq_in = nc.dram_tensor("bq_p", [P, M_CH], F32, kind="ExternalInput").ap()
    bk_in = nc.dram_tensor("bk_p", [P, M_CH], F32, kind="ExternalInput").ap()
    bv_in = nc.dram_tensor("bv_r", [P, DH], F32, kind="ExternalInput").ap()
    dmask_in = nc.dram_tensor("dmask", [P, KC], DT,
                              kind="ExternalInput").ap()
    ones_v_in = nc.dram_tensor("ones_v", [P, N_KC, NH_G, 1], DT,
                               kind="ExternalInput").ap()
    out_p = nc.dram_tensor("out_p", [S, D], F32, kind="ExternalOutput").ap()

    with tile.TileContext(nc) as tc, ExitStack() as ctx:
        consts = ctx.enter_context(tc.tile_pool(name="consts", bufs=1))
        wpool = ctx.enter_context(tc.tile_pool(name="w", bufs=1))
        qkv = ctx.enter_context(tc.tile_pool(name="qkv", bufs=1))
        xpool = ctx.enter_context(tc.tile_pool(name="xp", bufs=2))
        probs_pool = ctx.enter_context(tc.tile_pool(name="probs", bufs=4))
        attn_pool = ctx.enter_context(tc.tile_pool(name="attn", bufs=9))
        gpool = ctx.enter_context(tc.tile_pool(name="gp", bufs=2))
        spool = ctx.enter_context(tc.tile_pool(name="sp", bufs=3))
        bpool = ctx.enter_context(tc.tile_pool(name="bp", bufs=4))
        aupool = ctx.enter_context(tc.tile_pool(name="aupool", bufs=13))
        outst = ctx.enter_context(tc.tile_pool(name="outst", bufs=2))
        # PSUM: sc 2x2 banks + pv 2x1 + o (proj/outproj) 2x1 = 8 banks
        sc_ps = ctx.enter_context(
            tc.tile_pool(name="sc_ps", bufs=2, space="PSUM"))
        pv_ps = ctx.enter_context(
            tc.tile_pool(name="pv_ps", bufs=2, space="PSUM"))
        o_ps = ctx.enter_context(
            tc.tile_pool(name="o_ps", bufs=2, space="PSUM"))

        # small consts on sync queue first
        bq_sb = consts.tile([P, M_CH], F32)
        nc.sync.dma_start(bq_sb, bq_in)
        bk_sb = consts.tile([P, M_CH], F32)
        nc.sync.dma_start(bk_sb, bk_in)
        bv_sb = consts.tile([P, DH], F32)
        nc.sync.dma_start(bv_sb, bv_in)
        tri_sb = consts.tile([P, KC], DT)
        nc.sync.dma_start(tri_sb, dmask_in)
        ones_c = consts.tile([1, 64], DT)
        nc.vector.memset(ones_c, 1.0)

        # weights: all on the scalar HWDGE queue (idle early)
        w_tiles = {}
        for name in ("wv", "wk", "wq"):
            w_tiles[name] = wpool.tile([P, E_CH, DH], DT, tag=name,
                                       name=name)
            nc.scalar.dma_start(w_tiles[name], {"wv": wv_t, "wk": wk_t,
                                                "wq": wq_t}[name])
        wo_sb = wpool.tile([P, M_CH, D], DT, tag="wo")
        nc.scalar.dma_start(wo_sb, wo_t)

        qT = qkv.tile([P, M_CH, S], DT, tag="qT")
        kT = qkv.tile([P, M_CH, S], DT, tag="kT")
        v_aug = qkv.tile([P, N_KC, NH_G, 65], DT, tag="v_aug")
        if use_bf16:
            nc.gpsimd.memset(v_aug[:, :, :, 64:65], 1.0)
        else:
            nc.gpsimd.dma_start(v_aug[:, :, :, 64:65], ones_v_in)

        # ---------------- attention building blocks ----------------
        def attn_m_group(qb, m, sums_g, mq_work):
            """scores+exp+PV+drain for one (qb, m); yields once per chunk
            so the caller can interleave other work into the PE queue."""
            n_kc = (qb + 1) * R if causal else N_KC
            pv_t = [pv_ps.tile([65, QB], F32, tag="pv", name=f"pv{hl}")
                    for hl in (0, 1)]

            def emit_pv(kc, pt, c0):
                for hl in (0, 1):
                    nc.tensor.matmul(
                        pv_t[hl][:, c0:QB],
                        lhsT=v_aug[:, kc, 2 * m + hl, :],
                        rhs=pt[:, hl, c0:],
                        start=(kc == 0), stop=(kc == n_kc - 1),
                    )

            prev = None
            for kc in range(n_kc):
                r = kc - (n_kc - R) if causal else -1
                c0 = KC * r if r > 0 else 0
                sc2 = sc_ps.tile([P, 2, QB], F32, tag="sc", name="sc2")
                for hl in (0, 1):
                    rows = slice(64 * hl, 64 * hl + 64)
                    nc.tensor.matmul(
                        sc2[:, hl, c0:QB],
                        lhsT=kT[rows, m, kc * KC:(kc + 1) * KC],
                        rhs=qT[rows, m, qb * QB + c0:(qb + 1) * QB],
                        start=True, stop=True,
                    )
                pt = probs_pool.tile([P, 2, QB], DT, tag="pt")
                nc.scalar.activation(pt[:, :, c0:], sc2[:, :, c0:QB],
                                     AF.Exp, scale=float(SCALE))
                if r >= 0:
                    for hl in (0, 1):
                        nc.gpsimd.tensor_tensor(
                            pt[:, hl, c0:c0 + KC], pt[:, hl, c0:c0 + KC],
                            tri_sb, ALU.mult)
                if prev is not None:
                    emit_pv(*prev)
                prev = (kc, pt, c0)
                yield
            emit_pv(*prev)
            attn_us = []
            for hl in (0, 1):
                sums_sb = spool.tile([65, QB], F32, tag="sums_sb",
                                     name=f"sums{hl}")
                nc.vector.tensor_copy(sums_sb[64:65, :],
                                      pv_t[hl][64:65, 0:QB])
                nc.gpsimd.dma_start(sums_g[2 * m + hl: 2 * m + hl + 1],
                                    sums_sb[64:65, :])
                attn_u = aupool.tile([64, QB], DT, tag="attn_u",
                                     name=f"attn_u{hl}")
                nc.vector.tensor_copy(attn_u, pv_t[hl][0:64, 0:QB])
                attn_us.append(attn_u)
            mq_work.append((m, attn_us))
            yield

        class QbState:
            def __init__(s, qb):
                s.qb = qb
                s.mq = []
                s.sums_g = gpool.tile([2 * M_CH, QB], F32, tag="sums_g",
                                      name=f"sums_g{qb}")
                s.recips_f = gpool.tile([2 * M_CH, QB], F32, tag="recips_f",
                                        name=f"rf{qb}")
                s.recips_g = gpool.tile([2 * M_CH, QB], DT, tag="recips_g",
                                        name=f"rg{qb}")
                # recip rows folded onto partition 0 (Q7 broadcast src must
                # start at partition 0/32/64/96)
                s.rg1 = gpool.tile([1, 2 * M_CH, QB], DT, tag="rg1",
                                   name=f"rg1_{qb}", bufs=1)
                s.attn_tiles = []

        def norm_outproj_gen(st):
            """Generator: recip, per-m normalize, per-ssub outproj+store."""
            nc.vector.reciprocal_approx_fast(st.recips_f, st.sums_g)
            nc.vector.tensor_copy(st.recips_g, st.recips_f)
            nc.gpsimd.dma_start(st.rg1, st.recips_g)
            yield
            for m, attn_us in st.mq:
                attn_m = attn_pool.tile([P, QB], DT, tag="attn",
                                        name="attn_m")
                for hl in (0, 1):
                    # broadcast recip row to 64 partitions with a K=1 matmul
                    # (lhsT/rhs at partition 0 — a legal PE base partition)
                    rb = o_ps.tile([64, QB], F32, tag="o", name="rb")
                    nc.tensor.matmul(rb, lhsT=ones_c,
                                     rhs=st.rg1[0:1, 2 * m + hl, :],
                                     start=True, stop=True)
                    nc.vector.tensor_tensor(
                        attn_m[64 * hl:64 * hl + 64, :], attn_us[hl],
                        rb, ALU.mult)
                st.attn_tiles.append(attn_m)
                yield
            for ssub in range(QB // P):
                stt = outst.tile([P, O_N, 512], F32, tag="st", name="stt")
                for nout in range(O_N):
                    pso = o_ps.tile([P, 512], F32, tag="o", name="pso")
                    for m in range(M_CH):
                        nc.tensor.matmul(
                            pso,
                            lhsT=st.attn_tiles[m][:, ssub * P:(ssub + 1) * P],
                            rhs=wo_sb[:, m, nout * 512:(nout + 1) * 512],
                            start=(m == 0), stop=(m == M_CH - 1),
                        )
                    nc.vector.tensor_copy(stt[:, nout], pso)
                r0 = st.qb * QB + ssub * P
                nc.sync.dma_start(out_p[r0:r0 + P, :],
                                  stt.rearrange("p a b -> p (a b)"))
                yield

        # ---------------- projections ----------------
        # V proj first (v_aug ready before any PV)
        for n in range(NB):
            xblk = xpool.tile([P, E_CH, QB], DT, tag="xv", name="xvblk")
            nc.sync.dma_start(xblk, xv_t[n])
            for sc in range(QB // P):
                ps = o_ps.tile([P, DH], F32, tag="o", name="psv")
                for e in range(E_CH):
                    nc.tensor.matmul(
                        ps,
                        lhsT=xblk[:, e, sc * P:(sc + 1) * P],
                        rhs=w_tiles["wv"][:, e, :],
                        start=(e == 0), stop=(e == E_CH - 1),
                    )
                kc = n * (QB // P) + sc
                nc.vector.tensor_tensor(
                    v_aug[:, kc, :, 0:64],
                    ps.rearrange("p (h e) -> p h e", h=NH_G),
                    bv_sb.rearrange("p (h e) -> p h e", h=NH_G),
                    ALU.add,
                )

        def kq_load(phase, n2):
            x_in = xk_t if phase == "k" else xq_t
            xblk = xpool.tile([P, E_CH, WB], DT, tag="x" + phase,
                              name="xblk")
            nc.sync.dma_start(xblk, x_in[n2])
            return xblk

        def kq_group(phase, n2, m, xblk):
            w_sb = w_tiles["wk" if phase == "k" else "wq"]
            b_sb = bk_sb if phase == "k" else bq_sb
            ps = o_ps.tile([P, WB], F32, tag="o", name="pskq")
            for e in range(E_CH):
                nc.tensor.matmul(
                    ps,
                    lhsT=w_sb[:, e, m * P:(m + 1) * P],
                    rhs=xblk[:, e, :],
                    start=(e == 0), stop=(e == E_CH - 1),
                )
            dstp = kT if phase == "k" else qT
            nc.vector.tensor_scalar_add(
                dstp[:, m, n2 * WB:(n2 + 1) * WB], ps, b_sb[:, m:m + 1])

        # k and q block 0 straight through; qb=0 needs only these
        for phase in ("k", "q"):
            xb = kq_load(phase, 0)
            for m in range(M_CH):
                kq_group(phase, 0, m, xb)

        # prologue attention generators for qb0 (and qb1 if present)
        states = {0: QbState(0)}

        def gen_qb(qb):
            for m in range(M_CH):
                yield from attn_m_group(qb, m, states[qb].sums_g,
                                        states[qb].mq)
        genA = gen_qb(0)
        if N_QB > 1:
            states[1] = QbState(1)
            genB = gen_qb(1)
        else:
            genB = iter(())

        def pull(g):
            try:
                next(g)
                return True
            except StopIteration:
                return False

        def next_filler():
            if not pull(genA):
                pull(genB)

        # remaining k/q blocks, one prologue chunk per psum group
        for n2 in range(1, NW):
            for phase in ("k", "q"):
                xb = kq_load(phase, n2)
                for m in range(M_CH):
                    kq_group(phase, n2, m, xb)
                    next_filler()

        # finish qb0 attention (usually already done), then interleave
        # qb0 norm+outproj with the qb1 attention drain
        for _ in genA:
            pass
        if N_QB > 1:
            seq = norm_outproj_gen(states[0])
            alive = True
            while alive:
                alive = False
                for _ in range(3):
                    if pull(genB):
                        alive = True
                if pull(seq):
                    alive = True

        # ---------------- steady state: qb = 2.. ----------------
        for qb in range(2, N_QB):
            states[qb] = QbState(qb)
            seq = norm_outproj_gen(states[qb - 1])
            for m in range(M_CH):
                for _ in attn_m_group(qb, m, states[qb].sums_g,
                                      states[qb].mq):
                    pass
                pull(seq)
                pull(seq)
                if m >= 2:
                    pull(seq)
            for _ in seq:
                pass

        # tail: last qb's norm + outproj
        for _ in norm_outproj_gen(states[N_QB - 1]):
            pass
    nc.compile()
    return nc


def make_consts(S, use_bf16):
    """Host-built 0/1 upper-triangular (key<=query) mask for diag chunks."""
    npdt = _np_dt(use_bf16)
    i = np.arange(P)[:, None]
    j = np.arange(KC)[None, :]
    return (i <= j).astype(npdt)


def core_inputs(Q, K, V, W_q, b_q, W_k, b_k, W_v, b_v, W_o, b, hg, S, use_bf16):
    """Build the per-core input map (host-side slicing/transposition/casts)."""
    npdt = _np_dt(use_bf16)
    d = _dims(S)
    M_CH = d["M_CH"]
    rows = slice(hg * DH, (hg + 1) * DH)

    QB = d["QB"]
    E_CH = D // P

    def xt(x):
        # [S, D] -> [N_QB, P, E_CH, QB]: per-core x, transposed and tiled so
        # each device block load is a contiguous DMA.
        a = np.asarray(x, np.float32).T.astype(npdt)      # [D, S]
        a = a.reshape(E_CH, P, S // QB, QB).transpose(2, 1, 0, 3)
        return np.ascontiguousarray(a)

    def wt(w):
        # [DH, D] slice -> W^T tiled [P, E_CH, DH]
        a = np.asarray(w, np.float32).T.astype(npdt)      # [D, DH]
        return np.ascontiguousarray(
            a.reshape(E_CH, P, DH).transpose(1, 0, 2))

    a_wo = np.asarray(W_o[:, rows], np.float32).T.astype(npdt)  # [DH, D]
    wo_prep = np.ascontiguousarray(
        a_wo.reshape(M_CH, P, D).transpose(1, 0, 2))

    dmask = make_consts(S, use_bf16)
    return {
        "xq_t": xt(Q[b]), "xk_t": xt(K[b]), "xv_t": xt(V[b]),
        "wq_t": wt(W_q[rows]), "wk_t": wt(W_k[rows]), "wv_t": wt(W_v[rows]),
        "wo_t": wo_prep,
        "bq_p": np.ascontiguousarray(
            np.asarray(b_q[rows], np.float32).reshape(M_CH, P).T),
        "bk_p": np.ascontiguousarray(
            np.asarray(b_k[rows], np.float32).reshape(M_CH, P).T),
        "bv_r": np.broadcast_to(
            np.asarray(b_v[rows], np.float32), (P, DH)).copy(),
        "dmask": dmask,
        "ones_v": np.ones((P, d["N_KC"], NH_G, 1), npdt),
    }


def _np_reference(Q, K, V, mask, W_q, b_q, W_k, b_k, W_v, b_v, W_o, b_o):
    """Exact numpy fallback for arbitrary masks."""
    q = (Q @ W_q.T + b_q).reshape(B, S_FULL, H, DK).transpose(0, 2, 1, 3)
    k = (K @ W_k.T + b_k).reshape(B, S_FULL, H, DK).transpose(0, 2, 1, 3)
    v = (V @ W_v.T + b_v).reshape(B, S_FULL, H, DK).transpose(0, 2, 1, 3)
    scores = np.einsum("bhqd,bhkd->bhqk", q, k) / np.sqrt(np.float32(DK))
    scores = np.where(mask == 0, np.finfo(np.float32).min, scores)
    scores -= scores.max(-1, keepdims=True)
    probs = np.exp(scores)
    probs /= probs.sum(-1, keepdims=True)
    out = np.einsum("bhqk,bhkd->bhqd", probs, v)
    out = out.transpose(0, 2, 1, 3).reshape(B, S_FULL, D)
    return (out @ W_o.T + b_o).astype(np.float32)


def kernel(Q, K, V, mask, W_q, b_q, W_k, b_k, W_v, b_v, W_o, b_o):
    Q = np.asarray(Q, np.float32)
    K = np.asarray(K, np.float32)
    V = np.asarray(V, np.float32)
    mask = np.asarray(mask)

    m2 = mask.reshape(mask.shape[-2], mask.shape[-1])
    if np.array_equal(m2 != 0, np.tril(np.ones(m2.shape, bool))):
        causal = True
    elif (m2 != 0).all():
        causal = False
    else:
        return _np_reference(Q, K, V, mask, W_q, b_q, W_k, b_k, W_v, b_v,
                             W_o, b_o)

    use_bf16 = os.environ.get("MHA_KERNEL_DTYPE", "bf16") == "bf16"
    from concourse.bass_utils import run_bass_kernel_spmd

    key = (causal, S_FULL, use_bf16)
    if key not in _PROG_CACHE:
        _PROG_CACHE[key] = build_program(causal, S_FULL, use_bf16)
    nc = _PROG_CACHE[key]

    in_maps = []
    for c in range(8):
        b, hg = divmod(c, 2)
        in_maps.append(core_inputs(Q, K, V, W_q, b_q, W_k, b_k, W_v, b_v,
                                   W_o, b, hg, S_FULL, use_bf16))

    trace = os.environ.get("MHA_KERNEL_TRACE", "0") == "1"
    kw = {}
    if trace:
        kw = {"trace": True,
              "trace_cores": [int(x) for x in os.environ.get(
                  "MHA_TRACE_CORES", "0").split(",")]}
    n_cores = int(os.environ.get("MHA_CORES", "8"))
    res = run_bass_kernel_spmd(nc, in_maps[:n_cores],
                               core_ids=list(range(n_cores)), **kw)
    kernel.last_results = res

    b_o32 = np.asarray(b_o, np.float32)
    out = np.zeros((B, S_FULL, D), np.float32)
    for b in range(B):
        if 2 * b + 1 < n_cores:
            out[b] = (res.results[2 * b]["out_p"]
                      + res.results[2 * b + 1]["out_p"] + b_o32[None, :])
    return out


kernel.last_results = None
